# revision 1
# baseline (speedup 1.0000x reference)
"""Trainium2 Bass kernel for nn_F_VAE_can_7902739824969.

Reference, per batch row b with domain d = dom[b]:
    out[b] = F_d @ eps[b] + concat(bias_shared, bias_nonshared[d])
with F_d = (I - L_d)^{-1} S_d, L_d strictly-lower only in the last K=64 rows,
S_d diagonal.  Hence F_d = [[I, 0], [F21_d, F22_d]]: only the bottom K rows
(F_bot, [D, K, N]) carry information:
    out[b, :N-K] = eps[b, :N-K] + bias_shared
    out[b, N-K:] = F_bot[d] @ eps[b] + bias_nonshared[d]

Host (inside kernel()): solve the D unit-triangular systems for F_bot, sort
batch rows by domain, give each of 8 cores 128 sorted rows plus the few
domain blocks of F^T that shard touches.  Everything ships as bf16 (the
correctness gate is rel 2e-2; bf16 keeps us ~3e-3).

Device, per core (raw bacc).  All input DMAs are issued in `main`, before
the kernel block, so descriptor generation overlaps block entry; the tiny
bias blob rides the gpsimd SWDGE queue so it never serializes a HWDGE
queue.  PE does ALL the math into two PSUM banks:
    p_top [128,448]: per-chunk identity matmuls (epsT_c^T @ I, the PE
        doubles as the eps transpose engine; I is built on-device with one
        affine_select) + a rank-1 ones x bias_top matmul
    p_bot [128,K*nseg]: per-chunk epsT_c^T @ F^T chunks + rank-1 ones x
        bbot matmul (nonshared bias, segment-interleaved)
DVE casts p_top -> out cols 0:448, then selects the right segment of
p_bot per batch row with copy_predicated (segment masks are 0/1 per row =
per partition); one DMA ships the whole [128,512] bf16 output.
Cheap bf16 dummy matmuls bridge the PE clock-gate (HAM) from program
start to the first real matmul.  The whole program is straight-line in
main (no Bass block): all cross-engine ordering is semaphore-based, so the
block entry branches and end barrier would only delay each engine's entry
into the runtime epilogue (whose per-engine semaphore-clear ladder - ~6us
on Tensor - dominates the measured tail).
"""

import numpy as np
import ml_dtypes

B = 1024
N = 512
K = 64
D = 16
P = 128
NC = 8
RPC = B // NC          # rows per core
NTOP = N - K           # 448
NCHUNK = N // P        # 4 contraction chunks

BF16 = ml_dtypes.bfloat16

# PE keep-warm dummy matmuls (256-wide moving operand): W_START of them run
# while the input DMAs are in flight, bridging the PE from program start to
# the first real matmul so the HAM clock-gate opens.  The bridge must be
# continuous - a >1us idle gap re-cools the PE and the real matmuls run ~2x
# slower.  No tail dummies: each engine's section of the runtime's
# end-of-NEFF semaphore-clear ladder starts at that engine's retirement, and
# delaying Tensor's retirement costs exactly what the warmer clear cadence
# buys back.
W_START = 11
W_TAIL = 0

_PROG_CACHE: dict = {}


def _build_fbot(L_emb, S_emb):
    """F_bot [D, K, N] (float32): bottom K rows of (I - L_d)^{-1} S_d."""
    L_emb = np.asarray(L_emb, np.float64)
    S_emb = np.asarray(S_emb, np.float64)
    off = np.zeros(K, dtype=np.int64)
    for r in range(1, K):
        off[r] = off[r - 1] + (NTOP + r - 1)
    L21 = np.zeros((D, K, NTOP))
    L22 = np.zeros((D, K, K))
    for r in range(K):
        L21[1:, r, :] = L_emb[1:, off[r] : off[r] + NTOP]
        if r > 0:
            L22[1:, r, :r] = L_emb[1:, off[r] + NTOP : off[r] + NTOP + r]
    s = np.ones((D, K))
    s[1:] = S_emb[1:]
    rhs = np.concatenate([L21, s[:, :, None] * np.eye(K)[None]], axis=2)  # [D,K,N]
    X = np.zeros_like(rhs)
    for r in range(K):
        X[:, r, :] = rhs[:, r, :] + np.einsum(
            "dj,djn->dn", L22[:, r, :r], X[:, :r, :]
        )
    return X.astype(np.float32)


def _build_program(nseg):
    import concourse.bacc as bacc
    import concourse.mybir as mybir

    f32 = mybir.dt.float32
    bf16 = mybir.dt.bfloat16

    aw = NCHUNK * P              # epsT chunks
    bw = NCHUNK * nseg * K       # F^T chunks, (k*nseg+s)-interleaved
    rw = NTOP + nseg * K + P     # bias_top | bbot_flat | ones

    nc = bacc.Bacc()
    a_in = nc.declare_dram_parameter("a", [P, aw], bf16, isOutput=False)
    b_in = nc.declare_dram_parameter("b", [P, bw], bf16, isOutput=False)
    r_in = nc.declare_dram_parameter("r", [2, rw], bf16, isOutput=False)
    m_in = nc.declare_dram_parameter("m", [RPC, nseg], mybir.dt.uint8, isOutput=False)
    o_ext = nc.declare_dram_parameter("o", [RPC, N], bf16, isOutput=True)

    a_sb = nc.alloc_sbuf_tensor("a_sb", [P, aw], bf16).ap()
    b_sb = nc.alloc_sbuf_tensor("b_sb", [P, bw], bf16).ap()
    r_sb = nc.alloc_sbuf_tensor("r_sb", [2, rw], bf16).ap()
    m_sb = nc.alloc_sbuf_tensor("m_sb", [P, nseg], mybir.dt.uint8).ap()
    eye = nc.alloc_sbuf_tensor("eye", [P, P], bf16).ap()
    junk = nc.alloc_sbuf_tensor("junk", [P, NTOP], bf16).ap()
    out_sb = nc.alloc_sbuf_tensor("out_sb", [P, N], bf16).ap()

    p_top = nc.alloc_psum_tensor("p_top", [P, NTOP], f32).ap()
    p_bot = nc.alloc_psum_tensor("p_bot", [P, K, nseg], f32).ap()
    p_scr = nc.alloc_psum_tensor("p_scr", [P, 256], f32).ap()

    ones = r_sb[:, NTOP + nseg * K :]
    bias_top = r_sb[:, :NTOP]
    bbot = r_sb[:, NTOP : NTOP + nseg * K]

    s_junk = nc.alloc_semaphore("s_junk")
    s_a = nc.alloc_semaphore("s_a")
    s_b = nc.alloc_semaphore("s_b")
    s_r = nc.alloc_semaphore("s_r")
    s_m = nc.alloc_semaphore("s_m")
    s_eye = nc.alloc_semaphore("s_eye")
    s_pt = nc.alloc_semaphore("s_pt")
    s_pe = nc.alloc_semaphore("s_pe")
    s_bot = nc.alloc_semaphore("s_bot")
    s_out = nc.alloc_semaphore("s_out")

    one_bf16 = nc.const_aps.aps[(bf16, 1.0)]

    # ---- main: all input DMAs + on-device constants, before block entry ----
    nc.sync.dma_start(a_sb, a_in[:]).then_inc(s_a, 16)
    nc.scalar.dma_start(b_sb, b_in[:]).then_inc(s_b, 16)
    nc.scalar.dma_start(m_sb, m_in[:]).then_inc(s_m, 16)
    nc.gpsimd.memset(junk, 0).then_inc(s_junk, 1)
    # eye[p, n] = 1.0 where n - p == 0
    nc.gpsimd.affine_select(
        eye,
        one_bf16.to_broadcast([P, P]),
        pattern=[[1, P]],
        compare_op=mybir.AluOpType.is_equal,
        fill=0.0,
        base=0,
        channel_multiplier=-1,
    ).then_inc(s_eye, 1)
    nc.gpsimd.dma_start(r_sb, r_in[:]).then_inc(s_r, 16)  # SWDGE

    # ---- engine streams, straight-line in main: no Bass block.  All
    # cross-engine ordering is semaphore-based, and the runtime's own
    # end-of-NEFF barrier + queue drains follow the program anyway, so the
    # Bass block entry branches and end barrier would only add ~0.5us.
    te = nc.tensor
    # warm-up dummies may read garbage (scratch psum, never read back)
    for _ in range(W_START):
        te.matmul(p_scr[:16, :], lhsT=junk[:, :16], rhs=junk[:, :256],
                  start=True, stop=True)
    # p_top group opener: zeros from the junk buffer.  start=True resets
    # has_written for the whole bank, so the opener must be the
    # input-independent matmul, not the bias one.
    te.wait_ge(s_junk, 1)
    te.matmul(p_top, lhsT=junk[:, :P], rhs=junk[:, :NTOP],
              start=True, stop=False)
    # eps transpose: p_top[:, 128c:...] += epsT_c^T @ I
    te.wait_ge(s_a, 16)
    te.wait_ge(s_eye, 1)
    for c in range(NCHUNK):
        w = P if c < NCHUNK - 1 else P - K
        te.matmul(
            p_top[:, c * P : c * P + w],
            lhsT=a_sb[:, c * P : (c + 1) * P],
            rhs=eye[:, :w],
            start=False, stop=False,
        )
    # rank-1 bias fill: p_top += 1 (x) bias_top
    te.wait_ge(s_r, 16)
    te.matmul(p_top, lhsT=ones, rhs=bias_top,
              start=False, stop=True).then_inc(s_pt, 1)
    # bottom: p_bot = sum_c epsT_c^T @ F^T_c + 1 (x) bbot
    te.wait_ge(s_b, 16)
    for c in range(NCHUNK):
        te.matmul(
            p_bot,
            lhsT=a_sb[:, c * P : (c + 1) * P],
            rhs=b_sb[:, c * nseg * K : (c + 1) * nseg * K],
            start=(c == 0), stop=False,
        )
    te.matmul(p_bot.rearrange("p k s -> p (k s)"),
              lhsT=ones, rhs=bbot, start=False, stop=True).then_inc(s_pe, 1)
    for _ in range(W_TAIL):
        te.matmul(p_scr[:16, :], lhsT=junk[:, :16], rhs=junk[:, :256],
                  start=True, stop=True)

    ve = nc.vector
    ve.wait_ge(s_pt, 1)
    ve.tensor_copy(out_sb[:, :NTOP], p_top)
    ve.wait_ge(s_pe, 1)
    ve.wait_ge(s_m, 16)
    mm = None
    for s in range(nseg):
        mm = ve.copy_predicated(
            out_sb[:, NTOP:],
            m_sb[:, s, None].to_broadcast([P, K]),
            p_bot[:, :, s],
        )
    mm.then_inc(s_bot, 1)

    sy = nc.sync
    sy.wait_ge(s_bot, 1)
    sy.dma_start(o_ext[:], out_sb).then_inc(s_out, 16)

    nc.compile()
    return nc


def _prepare(epsilon, d, L_emb, S_emb, bias_nonshared, bias_shared):
    """Host-side sharding. Returns (nseg, in_maps, perm)."""
    eps = np.ascontiguousarray(np.asarray(epsilon, np.float32))
    dv = np.asarray(d).astype(np.int64).reshape(B)
    bias_ns = np.asarray(bias_nonshared, np.float32)
    bias_sh = np.asarray(bias_shared, np.float32).reshape(NTOP)

    fbot = _build_fbot(L_emb, S_emb)                     # [D, K, N]

    perm = np.argsort(dv, kind="stable")
    ds_sorted = dv[perm]
    eps_sorted = eps[perm]

    shard_segs = []
    for c in range(NC):
        rows = ds_sorted[c * RPC : (c + 1) * RPC]
        segs = []
        for dd in rows:
            if not segs or segs[-1] != dd:
                segs.append(int(dd))
        shard_segs.append(segs)
    nseg = max(len(s) for s in shard_segs)
    assert nseg <= 8, f"p_bot must fit one PSUM bank, got nseg={nseg}"

    rw = NTOP + nseg * K + P
    in_maps = []
    for c in range(NC):
        segs = shard_segs[c]
        rows = ds_sorted[c * RPC : (c + 1) * RPC]
        eps_c = eps_sorted[c * RPC : (c + 1) * RPC]          # [128, 512]

        # a: epsT chunks.  a[p, cc*128 + r] = eps[r, cc*128 + p]
        est = eps_c.T.reshape(NCHUNK, P, RPC)                # [cc, p, r]
        a = np.ascontiguousarray(
            est.transpose(1, 0, 2).reshape(P, NCHUNK * P)
        )

        # b: F^T chunks, col (cc, k, s) -> fbot[dom_s, k, cc*128+p]
        b = np.zeros((P, NCHUNK, K, nseg), np.float32)
        for s, dd in enumerate(segs):
            b[:, :, :, s] = fbot[dd].T.reshape(NCHUNK, P, K).transpose(1, 0, 2)
        b = b.reshape(P, NCHUNK * nseg * K)

        # r: bias_top | bbot_flat | ones (row 0 data, row 1 zeros; the
        # rank-1 matmuls use contraction dim 2 with ones on both rows)
        r = np.zeros((2, rw), np.float32)
        r[0, :NTOP] = bias_sh
        for s, dd in enumerate(segs):
            r[0, NTOP + np.arange(K) * nseg + s] = bias_ns[dd]
        r[:, NTOP + nseg * K :] = 1.0

        m = np.zeros((RPC, nseg), np.uint8)
        for s, dd in enumerate(segs):
            m[:, s] = (rows == dd).astype(np.uint8)

        in_maps.append({
            "a": a.astype(BF16),
            "b": b.astype(BF16),
            "r": r.astype(BF16),
            "m": m,
        })
    return nseg, in_maps, perm


def _finish(results, perm):
    out_sorted = np.concatenate(
        [np.asarray(results[c]["o"], dtype=np.float32) for c in range(NC)], axis=0
    )
    out = np.empty((B, N), np.float32)
    out[perm] = out_sorted
    return out


def get_program(nseg):
    prog = _PROG_CACHE.get(nseg)
    if prog is None:
        prog = _build_program(nseg)
        _PROG_CACHE[nseg] = prog
    return prog


def kernel(epsilon, d, L_emb, S_emb, bias_nonshared, bias_shared):
    from concourse.bass_utils import run_bass_kernel_spmd

    nseg, in_maps, perm = _prepare(
        epsilon, d, L_emb, S_emb, bias_nonshared, bias_shared
    )
    prog = get_program(nseg)
    res = run_bass_kernel_spmd(prog, in_maps, list(range(NC))).results
    return _finish(res, perm)

